# revision 12
# baseline (speedup 1.0000x reference)
"""Trainium2 Bass kernel for ColorGNNEmbedding (3-layer GCN, N=50000, E=800000).

Sharding: nodes split evenly across 8 NeuronCores; edges partitioned by
destination core. Per GCN layer:
  stage1: hW = h_local @ W + (layer-1: folded embedding vector)
  chunked AllGather of hW slabs -> full gather table ft (f16), tile-major
  agg:    per dst tile of 128 nodes, dma_gather the per-edge source rows
          (2 halves, exact per-bucket chunk counts, spread over 4 SWDGE
          queues) and reduce chunks with ON-CHIP-BUILT one-hot*norm S
          matrices on the TensorEngine, accumulating in PSUM. S chunks are
          built per tile by the Vector engine from a compact (dstcol, w)
          table via is_equal against an iota-replicated constant.
          Self-loops use a direct (contiguous) slab load + diagonal S chunk.
  batchnorm: per-feature sums/sumsq via ones-matmul, AllReduce, normalize +
          leaky relu, then PE-transpose each tile and immediately run the
          next layer's stage1 matmul (no DRAM round trip, no DMA transpose).
Weights / embedding tables are replicated; all five embedding lookups are
folded on the host into a single per-node vector added after stage1_c1.
"""

import math
import numpy as np
from contextlib import ExitStack
from dataclasses import dataclass

# ---------------------------------------------------------------------------
# configuration
# ---------------------------------------------------------------------------

P = 128
F_RES = 1000  # resnet feature count
F_IN = 1024   # padded contraction dim for stage-1 matmul
F1, F2, F3, FO = 512, 256, 64, 3
EPS = 1e-5
ALPHA = 0.01  # leaky relu slope
N_AG_CHUNKS = 4
NQ = 4        # SWDGE queues


@dataclass
class Cfg:
    n: int = 50000
    n_cores: int = 8
    # per-core edge layout metadata (identical shapes required across cores:
    # we pad per-core so all cores share one program)
    kth: tuple = ()      # [(K_t0, K_t1)] * NT, shared across cores (max)
    idx_off: tuple = ()  # idx col offset per (t, h)
    ch_off: tuple = ()   # chunk col offset per tile
    idxcols: int = 0
    chtot: int = 0

    @property
    def nloc(self):
        return self.n // self.n_cores

    @property
    def ntiles(self):
        return (self.nloc + P - 1) // P

    @property
    def npad(self):
        return self.ntiles * P

    @property
    def ag_bounds(self):
        nt = self.ntiles
        step = (nt + N_AG_CHUNKS - 1) // N_AG_CHUNKS
        return list(range(0, nt, step)) + [nt]


# ---------------------------------------------------------------------------
# host-side preprocessing (sharding / edge partitioning / weight folding)
# ---------------------------------------------------------------------------

def prep_edges(cfg: Cfg, edge_index, edge_attr):
    """Partition edges by destination core, bucket by (dst tile, src half).
    Emit (per core): wrapped-int16 gather indices (flat, exact per-bucket
    chunk counts shared across cores) plus compact per-chunk (dstcol, w)
    tables for on-chip S construction (last chunk per tile = self-loop)."""
    n, nloc, nt = cfg.n, cfg.nloc, cfg.ntiles
    NC = cfg.n_cores
    half_rows = nt * NC * P // 2
    src = np.asarray(edge_index[0], np.int64)
    dst = np.asarray(edge_index[1], np.int64)
    ew = np.asarray(edge_attr, np.float64)
    deg = np.bincount(dst, weights=ew, minlength=n) + 1.0
    dis = 1.0 / np.sqrt(deg)
    w = (dis[src] * ew * dis[dst]).astype(np.float32)
    dis2 = (1.0 / deg).astype(np.float32)

    # ft table row for a source node, matching the chunked AllGather output:
    # chunk ci covers tiles [b0,b1); its output is [core, (b1-b0)*P] row-major
    tsrc, psrc = (src % nloc) // P, (src % nloc) % P
    agb = np.asarray(cfg.ag_bounds, np.int64)
    ci_of_t = np.searchsorted(agb, tsrc, side="right") - 1
    b0, b1 = agb[ci_of_t], agb[ci_of_t + 1]
    srow = (b0 * NC + (src // nloc) * (b1 - b0) + (tsrc - b0)) * P + psrc
    core = dst // nloc

    percore = []
    cnts = np.zeros((NC, nt * 2), np.int64)
    for c in range(NC):
        m = core == c
        d_loc = dst[m] - c * nloc
        sr = srow[m]
        half = sr // half_rows
        grp = (d_loc // P) * 2 + half
        order = np.argsort(grp, kind="stable")
        s_, g_ = (sr % half_rows)[order], grp[order]
        d_, w_ = d_loc[order], w[m][order]
        cnts[c] = np.bincount(g_, minlength=nt * 2)
        percore.append((s_, d_, w_, g_))

    # shared-across-cores chunk counts (max per bucket so one program fits all)
    kth = np.ceil(cnts.max(axis=0).reshape(nt, 2) / P).astype(np.int64)
    cfg.kth = tuple((int(a), int(b)) for a, b in kth)
    idx_off, ch_off = [], []
    io = 0
    for t in range(nt):
        idx_off.append((io, io + kth[t, 0] * 8))
        io += (kth[t, 0] + kth[t, 1]) * 8
    cfg.idx_off = tuple(idx_off)
    cfg.idxcols = io
    co = 0
    for t in range(nt):
        ch_off.append(co)
        co += int(kth[t, 0] + kth[t, 1]) + 1
    cfg.ch_off = tuple(ch_off)
    cfg.chtot = co

    packs = []
    for c in range(NC):
        s_, d_, w_, grp = percore[c]
        starts = np.concatenate([[0], np.cumsum(cnts[c])])[:-1]
        j = np.arange(len(d_)) - starts[grp]
        k, p = j // P, j % P
        tile, half = grp // 2, grp % 2
        idxf = np.zeros((16, io), np.int16)
        dstf = np.zeros((P, co), np.float16)
        wvf = np.zeros((P, co), np.float16)
        # chunk index within tile: half0 chunks, then half1, then self
        coff = np.asarray(ch_off)[tile] + k + np.where(half == 1, kth[tile, 0], 0)
        dstf[p, coff] = (d_ % P).astype(np.float16)
        wvf[p, coff] = w_.astype(np.float16)
        ar = np.arange(P)
        for t in range(nt):
            dstf[:, ch_off[t] + kth[t, 0] + kth[t, 1]] = ar.astype(np.float16)
        d2 = np.zeros(nt * P, np.float32)
        d2[:nloc] = dis2[c * nloc:(c + 1) * nloc]
        wvf[:, np.asarray(ch_off) + kth[:, 0] + kth[:, 1]] = \
            d2.reshape(nt, P).astype(np.float16).T
        # wrapped int16 gather indices, flat cols
        iw = np.zeros((16, io), np.int16)
        icol = (np.asarray([o for o, _ in idx_off])[tile]
                + np.where(half == 1, kth[tile, 0] * 8, 0) + k * 8 + p // 16)
        iw[p % 16, icol] = s_.astype(np.int16)
        idxf = np.tile(iw, (8, 1))  # [128, io]
        packs.append((np.ascontiguousarray(idxf),
                      np.ascontiguousarray(dstf),
                      np.ascontiguousarray(wvf)))
    return packs


def prep_tables(layer_emb, size_emb, color_emb, W1):
    """Fold the embedding tables through W1 into one lookup table T[782, F1]."""
    W1 = np.asarray(W1, np.float64)
    le = np.asarray(layer_emb, np.float64)
    se = np.asarray(size_emb, np.float64)
    ce = np.asarray(color_emb, np.float64)
    return np.concatenate(
        [
            le @ W1[0:250],        # rows 0..2
            se @ W1[1250:1500],    # rows 3..13
            ce @ W1[1500:1585],    # rows 14..269
            ce @ W1[1585:1670],    # rows 270..525
            ce @ W1[1670:1755],    # rows 526..781
        ],
        axis=0,
    )  # [782, F1] float64


def prep_nodes(cfg: Cfg, x, T, c):
    """Per-core node inputs: tiled transposed resnet features (stage-1 lhsT)
    and the folded embedding vector femb = sum of 5 table lookups."""
    nloc, npad, nt = cfg.nloc, cfg.npad, cfg.ntiles
    xc = np.asarray(x[c * nloc:(c + 1) * nloc], np.float32)
    r = np.zeros((npad, F_IN), np.float16)
    r[:nloc, :F_RES] = xc[:, 1:1 + F_RES]
    r4 = r.reshape(nt, P, F_IN // P, P)  # [t, q, a, p]
    resnet_t = np.ascontiguousarray(r4.transpose(0, 3, 2, 1)).reshape(nt, P, F_IN)

    i0 = xc[:, 0].astype(np.int64)
    i1 = 3 + np.rint(xc[:, 1001] * 10).astype(np.int64)
    i2 = 14 + xc[:, 1002].astype(np.int64)
    i3 = 270 + xc[:, 1003].astype(np.int64)
    i4 = 526 + xc[:, 1004].astype(np.int64)
    femb = np.zeros((npad, F1), np.float16)
    femb[:nloc] = (T[i0] + T[i1] + T[i2] + T[i3] + T[i4]).astype(np.float16)
    return resnet_t, np.ascontiguousarray(femb.reshape(nt, P, F1))


def prep_weights(W1, W2, W3, Wo, bo):
    w1p = np.zeros((F_IN, F1), np.float16)
    w1p[:F_RES] = np.asarray(W1, np.float32)[250:1250]
    w1r = np.ascontiguousarray(w1p.reshape(F_IN // P, P, F1))
    w2r = np.ascontiguousarray(np.asarray(W2, np.float16).reshape(F1 // P, P, F2))
    w3r = np.ascontiguousarray(np.asarray(W3, np.float16).reshape(F2 // P, P, F3))
    wor = np.ascontiguousarray(np.asarray(Wo, np.float16))  # [64, 3]
    bor = np.asarray(bo, np.float16).reshape(1, FO)
    return w1r, w2r, w3r, wor, bor


def prep_inputs(cfg: Cfg, inputs):
    """Full host prep: returns in_maps for the 8 cores."""
    x = np.asarray(inputs["x"], np.float32)
    packs = prep_edges(cfg, inputs["edge_index"], inputs["edge_attr"])
    T = prep_tables(inputs["layer_emb"], inputs["size_emb"],
                    inputs["color_emb"], inputs["W1"])
    w1r, w2r, w3r, wor, bor = prep_weights(
        inputs["W1"], inputs["W2"], inputs["W3"], inputs["Wo"], inputs["bo"])
    gb1 = np.ascontiguousarray(np.stack([inputs["g1"], inputs["be1"]]).astype(np.float32))
    gb2 = np.ascontiguousarray(np.stack([inputs["g2"], inputs["be2"]]).astype(np.float32))
    gb3 = np.ascontiguousarray(np.stack([inputs["g3"], inputs["be3"]]).astype(np.float32))
    nchmax = max(cfg.kth[t][0] + cfg.kth[t][1] + 1 for t in range(cfg.ntiles))
    iotar = np.tile(np.arange(P, dtype=np.float16)[None, :], (P, nchmax))

    in_maps = []
    for c in range(cfg.n_cores):
        resnet_t, femb = prep_nodes(cfg, x, T, c)
        idxf, dstf, wvf = packs[c]
        in_maps.append({
            "resnet": resnet_t,
            "femb": femb,
            "idxf": idxf,
            "dstf": dstf,
            "wvf": wvf,
            "iotar": iotar,
            "w1": w1r, "w2": w2r, "w3": w3r, "wo": wor, "wob": bor,
            "gb1": gb1, "gb2": gb2, "gb3": gb3,
        })
    return in_maps


# ---------------------------------------------------------------------------
# device program
# ---------------------------------------------------------------------------

def build_program(cfg: Cfg):
    import concourse.bacc as bacc
    import concourse.tile as tile
    from concourse import mybir
    from concourse.masks import make_identity

    f16, f32, i16 = mybir.dt.float16, mybir.dt.float32, mybir.dt.int16
    f8 = mybir.dt.float8e4
    AF = mybir.ActivationFunctionType
    OP = mybir.AluOpType
    NT, NPAD, NC = cfg.ntiles, cfg.npad, cfg.n_cores
    KTH, IDXOFF, CHOFF = cfg.kth, cfg.idx_off, cfg.ch_off
    NCH = [KTH[t][0] + KTH[t][1] + 1 for t in range(NT)]
    NCHMAX = max(NCH)
    AGB = cfg.ag_bounds
    GROUPS = [list(range(NC))]

    nc = bacc.Bacc("TRN2", target_bir_lowering=False, debug=False,
                   num_devices=NC, num_swdge_queues=NQ)

    # --- parameters -------------------------------------------------------
    resnet_d = nc.dram_tensor("resnet", [NT, P, F_IN], f16, kind="ExternalInput")
    femb_d = nc.dram_tensor("femb", [NT, P, F1], f16, kind="ExternalInput")
    idxf_d = nc.dram_tensor("idxf", [P, cfg.idxcols], i16, kind="ExternalInput")
    dstf_d = nc.dram_tensor("dstf", [P, cfg.chtot], f16, kind="ExternalInput")
    wvf_d = nc.dram_tensor("wvf", [P, cfg.chtot], f16, kind="ExternalInput")
    iotar_d = nc.dram_tensor("iotar", [P, NCHMAX * P], f16, kind="ExternalInput")
    w1_d = nc.dram_tensor("w1", [F_IN // P, P, F1], f16, kind="ExternalInput")
    w2_d = nc.dram_tensor("w2", [F1 // P, P, F2], f16, kind="ExternalInput")
    w3_d = nc.dram_tensor("w3", [F2 // P, P, F3], f16, kind="ExternalInput")
    wo_d = nc.dram_tensor("wo", [64, FO], f16, kind="ExternalInput")
    wob_d = nc.dram_tensor("wob", [1, FO], f16, kind="ExternalInput")
    gb_d = [nc.dram_tensor(nm, [2, f], f32, kind="ExternalInput")
            for nm, f in (("gb1", F1), ("gb2", F2), ("gb3", F3))]
    out_d = nc.dram_tensor("out", [NPAD, FO], f32, kind="ExternalOutput")

    # --- internal DRAM ----------------------------------------------------
    Fs = [F1, F2, F3]
    Fg = [F1, F2, P]  # gather-table row widths (conv3 padded to 128)
    # f16 throughout the aggregation path: fp8 was measured at rel_err 0.066
    # (aggregation keeps relative noise: |Y| ~ sqrt(deg) like the noise sum).
    adt = [f16, f16, f16]
    slab_d = [nc.dram_tensor(f"slab{l}", [NPAD, Fg[l]], adt[l]) for l in range(3)]
    ft_d = [nc.dram_tensor(f"ft{l}", [NC * NPAD, Fg[l]], adt[l], addr_space="Shared")
            for l in range(3)]
    sin_d = [nc.dram_tensor(f"sin{l}", [1, 2 * Fs[l]], f32) for l in range(3)]
    sout_d = [nc.dram_tensor(f"sout{l}", [1, 2 * Fs[l]], f32, addr_space="Shared")
              for l in range(3)]

    with tile.TileContext(nc) as tc, ExitStack() as top:
        const = top.enter_context(tc.tile_pool(name="const", bufs=1))
        ident = const.tile([P, P], f16)
        make_identity(nc, ident[:])
        ones_col = const.tile([P, 1], f16)
        nc.vector.memset(ones_col[:], 1.0)
        ones_row = const.tile([1, P], f16)
        nc.vector.memset(ones_row[:], 1.0)

        w_sb = []
        for l, (wd, fin_t, fout) in enumerate(
                [(w1_d, F_IN // P, F1), (w2_d, F1 // P, F2), (w3_d, F2 // P, F3)]):
            wt = const.tile([P, fin_t * fout], f16, tag=f"w{l}")
            for a in range(fin_t):
                nc.sync.dma_start(out=wt[:, a * fout:(a + 1) * fout], in_=wd[a])
            w_sb.append(wt)
        wo_sb = const.tile([64, FO], f16)
        nc.sync.dma_start(out=wo_sb[:], in_=wo_d[:])
        wob_sb = const.tile([1, FO], f16)
        nc.sync.dma_start(out=wob_sb[:], in_=wob_d[:])
        # gather indices / chunk tables / iota, resident for the whole kernel
        idx_sb = const.tile([P, cfg.idxcols], i16, tag="idx")
        nc.sync.dma_start(out=idx_sb[:], in_=idxf_d[:])
        dst_sb = const.tile([P, cfg.chtot], f16, tag="dst")
        nc.sync.dma_start(out=dst_sb[:], in_=dstf_d[:])
        wv_sb = const.tile([P, cfg.chtot], f16, tag="wv")
        nc.sync.dma_start(out=wv_sb[:], in_=wvf_d[:])
        iota_sb = const.tile([P, NCHMAX * P], f16, tag="iota")
        nc.sync.dma_start(out=iota_sb[:], in_=iotar_d[:])

        gq = [0]  # round-robin SWDGE queue assignment

        def allgather_chunks(l, scope):
            with nc.named_scope(scope):
                for ci in range(len(AGB) - 1):
                    r0, r1 = AGB[ci] * P, AGB[ci + 1] * P
                    nc.gpsimd.collective_compute(
                        "AllGather", mybir.AluOpType.bypass,
                        replica_groups=GROUPS,
                        ins=[slab_d[l][r0:r1, :]],
                        outs=[ft_d[l][r0 * NC:r1 * NC, :]])

        def agg_layer(ctx, actx, l, F):
            """Edge aggregation for layer l -> hpre [P, NT*F] f16 + stats psums."""
            FG = Fg[l]
            dt = adt[l]
            HR = NC * NPAD // 2  # rows per half table
            hp_pool = ctx.enter_context(tc.tile_pool(name=f"hpre{l}", bufs=1))
            hpre = hp_pool.tile([P, NT * F], f16)
            sp_pool = ctx.enter_context(tc.tile_pool(name=f"stat{l}", bufs=1, space="PSUM"))
            s_ps = sp_pool.tile([1, F], f32)
            q_ps = sp_pool.tile([1, F], f32)
            ap_pool = actx.enter_context(tc.tile_pool(name=f"aggp{l}", bufs=4, space="PSUM"))
            g_pool = actx.enter_context(tc.tile_pool(name=f"g{l}", bufs=3))
            s_pool = actx.enter_context(tc.tile_pool(name=f"s{l}", bufs=3))
            sq_pool = actx.enter_context(tc.tile_pool(name=f"sq{l}", bufs=3))
            sf_pool = actx.enter_context(tc.tile_pool(name=f"sf{l}", bufs=3))
            pending = []  # (t, hp_ap, sq_ap): stats lag one tile so the
            # in-order PE never waits on the Vector/Scalar hp/sq producers

            def emit_stats():
                tp, hp_, sq_ = pending.pop(0)
                nc.tensor.matmul(out=s_ps[:], lhsT=ones_col[:], rhs=hp_,
                                 start=(tp == 0), stop=(tp == NT - 1))
                nc.tensor.matmul(out=q_ps[:], lhsT=ones_col[:], rhs=sq_[:],
                                 start=(tp == 0), stop=(tp == NT - 1))

            for t in range(NT):
                k0, k1 = KTH[t]
                ne = k0 + k1  # edge chunks (self-loop handled on DVE)
                nch = NCH[t]
                # build S chunks on-chip: S[gpos, c*128+d] = w iff dstcol==d
                St = s_pool.tile([P, NCHMAX * P], dt, tag="S")
                Sv = St[:, 0:ne * P].rearrange("p (c d) -> p c d", c=ne)
                nc.vector.tensor_tensor(
                    out=Sv,
                    in0=dst_sb[:, CHOFF[t]:CHOFF[t] + ne]
                        .unsqueeze(2).to_broadcast([P, ne, P]),
                    in1=iota_sb[:, 0:ne * P].rearrange("p (c d) -> p c d", c=ne),
                    op=OP.is_equal)
                nc.vector.tensor_tensor(
                    out=Sv, in0=Sv,
                    in1=wv_sb[:, CHOFF[t]:CHOFF[t] + ne]
                        .unsqueeze(2).to_broadcast([P, ne, P]),
                    op=OP.mult)
                selfh = sf_pool.tile([P, FG], dt, tag="selfh")
                nc.sync.dma_start(out=selfh[:], in_=slab_d[l][t * P:(t + 1) * P, :])
                G = g_pool.tile([P, ne * FG], dt, tag="G")
                for h, kh in ((0, k0), (1, k1)):
                    if kh == 0:
                        continue
                    off = 0 if h == 0 else k0 * FG
                    ioff = IDXOFF[t][h]
                    ksplit = [(0, kh)] if kh <= 1 else \
                        [(0, (kh + 1) // 2), ((kh + 1) // 2, kh - (kh + 1) // 2)]
                    for ko, kc in ksplit:
                        nc.gpsimd.dma_gather(
                            out_ap=G[:, off + ko * FG:off + (ko + kc) * FG]
                                .rearrange("p (k f) -> p k f", k=kc),
                            in_ap=ft_d[l][h * HR:(h + 1) * HR, :],
                            idxs_ap=idx_sb[:, ioff + ko * 8:ioff + (ko + kc) * 8],
                            num_idxs=kc * P, num_idxs_reg=kc * P, elem_size=FG,
                            single_packet=False, queue_num=gq[0] % NQ)
                        gq[0] += 1
                apsum = ap_pool.tile([P, F], f32, space="PSUM")
                for c in range(ne):
                    nc.tensor.matmul(out=apsum[:], lhsT=St[:, c * P:(c + 1) * P],
                                     rhs=G[:, c * FG:c * FG + F],
                                     start=(c == 0), stop=(c == ne - 1))
                if pending:
                    emit_stats()
                # self-loop + PSUM evacuation on the Vector engine:
                # hp = selfh * dis2 + apsum  (dis2 = self column of wv table)
                hp = hpre[:, t * F:(t + 1) * F]
                selfc = CHOFF[t] + nch - 1
                nc.vector.scalar_tensor_tensor(
                    out=hp, in0=selfh[:, 0:F],
                    scalar=wv_sb[:, selfc:selfc + 1], in1=apsum[:],
                    op0=OP.mult, op1=OP.add)
                sq = sq_pool.tile([P, F], f16)
                nc.scalar.square(sq[:], hp)
                pending.append((t, hp, sq))
            emit_stats()
            return hpre, s_ps, q_ps

        def stats_scales(ctx, l, F, s_ps, q_ps):
            """AllReduce sums/sumsq -> broadcast scale/shift tiles [P, F] f16."""
            sp = ctx.enter_context(tc.tile_pool(name=f"bns{l}", bufs=1))
            bp = ctx.enter_context(tc.tile_pool(name=f"bnp{l}", bufs=1, space="PSUM"))
            ssb = sp.tile([1, 2 * F], f32)
            nc.vector.tensor_copy(ssb[:, 0:F], s_ps[:])
            nc.vector.tensor_copy(ssb[:, F:2 * F], q_ps[:])
            nc.sync.dma_start(out=sin_d[l][:], in_=ssb[:])
            nc.gpsimd.collective_compute(
                "AllReduce", mybir.AluOpType.add, replica_groups=GROUPS,
                ins=[sin_d[l][:]], outs=[sout_d[l][:]])
            srep = sp.tile([1, 2 * F], f32)
            nc.sync.dma_start(out=srep[:], in_=sout_d[l][:])
            gsb = sp.tile([1, F], f32)
            nc.sync.dma_start(out=gsb[:], in_=gb_d[l][0:1, :])
            bsb = sp.tile([1, F], f32)
            nc.sync.dma_start(out=bsb[:], in_=gb_d[l][1:2, :])
            mean = sp.tile([1, F], f32)
            nc.vector.tensor_scalar_mul(mean[:], srep[:, 0:F], 1.0 / cfg.n)
            var = sp.tile([1, F], f32)
            nc.vector.tensor_scalar_mul(var[:], srep[:, F:2 * F], 1.0 / cfg.n)
            m2 = sp.tile([1, F], f32)
            nc.vector.tensor_tensor(out=m2[:], in0=mean[:], in1=mean[:], op=OP.mult)
            nc.vector.tensor_tensor(out=var[:], in0=var[:], in1=m2[:], op=OP.subtract)
            nc.vector.tensor_scalar_add(var[:], var[:], EPS)
            rec = sp.tile([1, F], f32)
            nc.vector.reciprocal(rec[:], var[:])
            rs = sp.tile([1, F], f32)
            nc.scalar.sqrt(rs[:], rec[:])  # rsqrt(var+eps)
            sc = sp.tile([1, F], f32)
            nc.vector.tensor_tensor(out=sc[:], in0=gsb[:], in1=rs[:], op=OP.mult)
            sh = sp.tile([1, F], f32)
            nc.vector.tensor_tensor(out=sh[:], in0=mean[:], in1=sc[:], op=OP.mult)
            nc.vector.tensor_tensor(out=sh[:], in0=bsb[:], in1=sh[:], op=OP.subtract)
            sc16 = sp.tile([1, F], f16)
            nc.vector.tensor_copy(sc16[:], sc[:])
            sh16 = sp.tile([1, F], f16)
            nc.vector.tensor_copy(sh16[:], sh[:])
            scp = bp.tile([P, F], f32, space="PSUM")
            nc.tensor.matmul(out=scp[:], lhsT=ones_row[:], rhs=sc16[:],
                             start=True, stop=True)
            shp = bp.tile([P, F], f32, space="PSUM")
            nc.tensor.matmul(out=shp[:], lhsT=ones_row[:], rhs=sh16[:],
                             start=True, stop=True)
            screp = sp.tile([P, F], f16)
            nc.scalar.activation(screp[:], scp[:], AF.Copy)
            shrep = sp.tile([P, F], f16)
            nc.scalar.activation(shrep[:], shp[:], AF.Copy)
            return screp, shrep

        def norm_tile(np_pool, l, F, hpre, t, screp, shrep):
            hn = np_pool.tile([P, F], f16, tag="hn")
            nc.vector.tensor_tensor(out=hn[:], in0=hpre[:, t * F:(t + 1) * F],
                                    in1=screp[:], op=OP.mult)
            nc.vector.tensor_tensor(out=hn[:], in0=hn[:], in1=shrep[:], op=OP.add)
            ha = np_pool.tile([P, F], f16, tag="ha")
            nc.scalar.activation(ha[:], hn[:], AF.Copy, scale=ALPHA)
            nc.vector.tensor_tensor(out=hn[:], in0=hn[:], in1=ha[:], op=OP.max)
            return hn

        # ================= conv1 =================
        with ExitStack() as ctx:
            with nc.named_scope("stage1_c1"), ExitStack() as sctx:
                r_pool = sctx.enter_context(tc.tile_pool(name="res", bufs=3))
                e_pool = sctx.enter_context(tc.tile_pool(name="emb", bufs=3))
                p1_pool = sctx.enter_context(tc.tile_pool(name="p1", bufs=3, space="PSUM"))
                hw_pool = sctx.enter_context(tc.tile_pool(name="hw1", bufs=3))
                for t in range(NT):
                    rsb = r_pool.tile([P, F_IN], f16)
                    nc.sync.dma_start(out=rsb[:], in_=resnet_d[t])
                    fe = e_pool.tile([P, F1], f16, tag="fe")
                    nc.sync.dma_start(out=fe[:], in_=femb_d[t])
                    ps = p1_pool.tile([P, F1], f32, space="PSUM")
                    for a in range(F_IN // P):
                        nc.tensor.matmul(
                            out=ps[:], lhsT=rsb[:, a * P:(a + 1) * P],
                            rhs=w_sb[0][:, a * F1:(a + 1) * F1],
                            start=(a == 0), stop=(a == F_IN // P - 1))
                    hw = hw_pool.tile([P, F1], adt[0], tag="hw")
                    nc.vector.tensor_tensor(out=hw[:], in0=ps[:], in1=fe[:],
                                            op=mybir.AluOpType.add)
                    nc.sync.dma_start(out=slab_d[0][t * P:(t + 1) * P, :], in_=hw[:])
            allgather_chunks(0, "ag_c1")
            with nc.named_scope("agg_c1"), ExitStack() as actx:
                hpre, s_ps, q_ps = agg_layer(ctx, actx, 0, F1)
            with nc.named_scope("bn_c1"), ExitStack() as bctx:
                screp, shrep = stats_scales(bctx, 0, F1, s_ps, q_ps)
                np_pool = bctx.enter_context(tc.tile_pool(name="nrm1", bufs=3))
                tp_pool = bctx.enter_context(tc.tile_pool(name="tp1", bufs=2, space="PSUM"))
                ht_pool = bctx.enter_context(tc.tile_pool(name="ht1", bufs=3))
                p2_pool = bctx.enter_context(tc.tile_pool(name="p2", bufs=2, space="PSUM"))
                hw2_pool = bctx.enter_context(tc.tile_pool(name="hw2", bufs=3))
                for t in range(NT):
                    hn = norm_tile(np_pool, 0, F1, hpre, t, screp, shrep)
                    hT = ht_pool.tile([P, F1], f16, tag="hT")
                    for a in range(F1 // P):
                        pst = tp_pool.tile([P, P], f16, space="PSUM")
                        nc.tensor.transpose(out=pst[:], in_=hn[:, a * P:(a + 1) * P],
                                            identity=ident[:])
                        nc.scalar.activation(hT[:, a * P:(a + 1) * P], pst[:], AF.Copy)
                    ps2 = p2_pool.tile([P, F2], f32, space="PSUM")
                    for a in range(F1 // P):
                        nc.tensor.matmul(
                            out=ps2[:], lhsT=hT[:, a * P:(a + 1) * P],
                            rhs=w_sb[1][:, a * F2:(a + 1) * F2],
                            start=(a == 0), stop=(a == F1 // P - 1))
                    hw2 = hw2_pool.tile([P, F2], adt[1], tag="hw2")
                    nc.scalar.activation(hw2[:], ps2[:], AF.Copy)
                    nc.sync.dma_start(out=slab_d[1][t * P:(t + 1) * P, :], in_=hw2[:])
            allgather_chunks(1, "ag_c2")

        # ================= conv2 =================
        with ExitStack() as ctx:
            with nc.named_scope("agg_c2"), ExitStack() as actx:
                hpre, s_ps, q_ps = agg_layer(ctx, actx, 1, F2)
            with nc.named_scope("bn_c2"), ExitStack() as bctx:
                screp, shrep = stats_scales(bctx, 1, F2, s_ps, q_ps)
                np_pool = bctx.enter_context(tc.tile_pool(name="nrm2", bufs=3))
                tp_pool = bctx.enter_context(tc.tile_pool(name="tp2", bufs=2, space="PSUM"))
                ht_pool = bctx.enter_context(tc.tile_pool(name="ht2", bufs=3))
                p3_pool = bctx.enter_context(tc.tile_pool(name="p3", bufs=2, space="PSUM"))
                hw3_pool = bctx.enter_context(tc.tile_pool(name="hw3", bufs=3))
                for t in range(NT):
                    hn = norm_tile(np_pool, 1, F2, hpre, t, screp, shrep)
                    hT = ht_pool.tile([P, F2], f16, tag="hT")
                    for a in range(F2 // P):
                        pst = tp_pool.tile([P, P], f16, space="PSUM")
                        nc.tensor.transpose(out=pst[:], in_=hn[:, a * P:(a + 1) * P],
                                            identity=ident[:])
                        nc.scalar.activation(hT[:, a * P:(a + 1) * P], pst[:], AF.Copy)
                    ps3 = p3_pool.tile([P, F3], f32, space="PSUM")
                    for a in range(F2 // P):
                        nc.tensor.matmul(
                            out=ps3[:], lhsT=hT[:, a * P:(a + 1) * P],
                            rhs=w_sb[2][:, a * F3:(a + 1) * F3],
                            start=(a == 0), stop=(a == F2 // P - 1))
                    hw3 = hw3_pool.tile([P, P], f16, tag="hw3")
                    nc.vector.memset(hw3[:, F3:P], 0.0)
                    nc.scalar.activation(hw3[:, 0:F3], ps3[:], AF.Copy)
                    nc.sync.dma_start(out=slab_d[2][t * P:(t + 1) * P, :], in_=hw3[:])
            allgather_chunks(2, "ag_c3")

        # ================= conv3 + output =================
        with ExitStack() as ctx:
            with nc.named_scope("agg_c3"), ExitStack() as actx:
                hpre, s_ps, q_ps = agg_layer(ctx, actx, 2, F3)
            with nc.named_scope("bn_c3"), ExitStack() as bctx:
                screp, shrep = stats_scales(bctx, 2, F3, s_ps, q_ps)
                np_pool = bctx.enter_context(tc.tile_pool(name="nrm3", bufs=3))
                tp_pool = bctx.enter_context(tc.tile_pool(name="tp3", bufs=2, space="PSUM"))
                ht_pool = bctx.enter_context(tc.tile_pool(name="ht3", bufs=3))
                po_pool = bctx.enter_context(tc.tile_pool(name="po", bufs=2, space="PSUM"))
                o_pool = bctx.enter_context(tc.tile_pool(name="osb", bufs=3))
                for t in range(NT):
                    hn = norm_tile(np_pool, 2, F3, hpre, t, screp, shrep)
                    pst = tp_pool.tile([P, P], f16, space="PSUM")
                    nc.tensor.transpose(out=pst[0:F3, :], in_=hn[:],
                                        identity=ident[:])
                    hT = ht_pool.tile([F3, P], f16, tag="hT")
                    nc.scalar.activation(hT[:], pst[0:F3, :], AF.Copy)
                    pso = po_pool.tile([P, FO], f32, space="PSUM")
                    nc.tensor.matmul(out=pso[:], lhsT=hT[:], rhs=wo_sb[:],
                                     start=True, stop=False)
                    nc.tensor.matmul(out=pso[:], lhsT=ones_row[:], rhs=wob_sb[:],
                                     start=False, stop=True)
                    ot = o_pool.tile([P, FO], f32)
                    nc.scalar.activation(ot[:], pso[:], AF.Copy)
                    nc.sync.dma_start(out=out_d[t * P:(t + 1) * P, :], in_=ot[:])

    nc.compile()
    return nc


# ---------------------------------------------------------------------------
# entry point
# ---------------------------------------------------------------------------

def run(inputs, cfg=None, trace=False):
    from concourse.bass_utils import run_bass_kernel_spmd

    if cfg is None:
        cfg = Cfg(n=int(np.asarray(inputs["x"]).shape[0]))
    in_maps = prep_inputs(cfg, inputs)
    nc = build_program(cfg)
    res = run_bass_kernel_spmd(nc, in_maps, core_ids=list(range(cfg.n_cores)),
                               trace=trace)
    out = np.empty((cfg.n, FO), np.float32)
    for c in range(cfg.n_cores):
        out[c * cfg.nloc:(c + 1) * cfg.nloc] = res.results[c]["out"][:cfg.nloc]
    return out, res


def kernel(**inputs) -> np.ndarray:
    out, _ = run(inputs)
    return out
